# revision 19
# baseline (speedup 1.0000x reference)
import os

os.environ.setdefault("MYCRO_LOCAL_CACHE", "1")

import sys

if "/opt/trn_rl_repo" not in sys.path:
    sys.path.insert(0, "/opt/trn_rl_repo")

import numpy as np
import ml_dtypes

import concourse.bass as bass
import concourse.bacc as bacc
import concourse.tile as tile
from concourse import mybir
from concourse.bass_utils import run_bass_kernel_spmd

BF16 = ml_dtypes.bfloat16

B, N, DIN, DOUT, E, CHEB_K = 32, 500, 64, 64, 16, 3
NCORES = 8
BPC = B // NCORES  # batches per core
NT = 4  # node tiles per batch
NPAD = 512  # padded node count
NTS = [128, 128, 128, 116]  # valid rows per node tile (u2 contraction)

f32 = mybir.dt.float32
f32r = mybir.dt.float32r
bf16 = mybir.dt.bfloat16

_CACHE = {}

# xmisc free-layout offsets
XA_OFF = 0  # [128, 4, 65]  xa chunks ([x | valid])
S_OFF = 260  # [128, 4, 16]  station_emb chunks
XM_W = 260 + 64


def _build_program():
    """Per-core SPMD program (same for all cores).

    Math per batch (P = exp(relu(emb emb^T)) is symmetric; r = row sums):
      u1T[d,n] = sum_m xa[m,d] P[m,n]        (row 64 = r via ones col)
      x_g1T    = u1T * (1/r)                 -> gA rows 64:127
      x_g1n    = transpose(x_g1T)            (PE, identity rhs)
      u2T[d,n] = sum_m x_g1n[m,d] P[m,n]
      gB[0:64] = u2T * (1/r)                 (= A(Ax); Cheb algebra folded:
                                              W1_x := Wp0 - Wp2, W2 := 2*Wp2)
      y[n,(o,e)] = [xT;x_g1T].T W1 + [gB;1].T W2   (bias via ones row)
      out[n,o] = sum_e y[n,(o,e)] * s[n,e]   (bcast mult + halve/reduce)

    Software pipelined: scores/exp of batch b+1 are emitted inside batch
    b's slot so PE/ACT never wait on them; input DMAs prefetch 2 slots
    ahead; y W1-matmuls are hoisted before gB is needed.
    """
    if "nc" in _CACHE:
        return _CACHE["nc"]

    nc = bacc.Bacc(target_bir_lowering=False, trn_type="TRN2", debug=False)
    AF = mybir.ActivationFunctionType
    ALU = mybir.AluOpType

    embT_d = nc.dram_tensor("embT", [16, BPC * NPAD], f32r, kind="ExternalInput")
    xm_d = nc.dram_tensor("xmisc", [BPC, 128, XM_W], bf16, kind="ExternalInput")
    xT_d = nc.dram_tensor("xT", [BPC, 64, NPAD], bf16, kind="ExternalInput")
    ones_d = nc.dram_tensor("onesrow", [1, NPAD], bf16, kind="ExternalInput")
    W1_d = nc.dram_tensor("W1", [128, 1024], bf16, kind="ExternalInput")
    W2_d = nc.dram_tensor("W2", [65, 1024], bf16, kind="ExternalInput")
    id_d = nc.dram_tensor("ident", [128, 64], bf16, kind="ExternalInput")
    out_d = nc.dram_tensor("out", [BPC, 128, 256], bf16, kind="ExternalOutput")

    dbg = os.environ.get("KERNEL_DEBUG", "0") == "1"
    if dbg:
        dbgP_d = nc.dram_tensor("dbgP", [128, 2048], bf16, kind="ExternalOutput")
        dbgJu1_d = nc.dram_tensor("dbgJu1", [65, NPAD], f32, kind="ExternalOutput")
        dbgrB_d = nc.dram_tensor("dbgrB", [64, NPAD], f32, kind="ExternalOutput")
        dbggA_d = nc.dram_tensor("dbggA", [128, NPAD], bf16, kind="ExternalOutput")
        dbgx1n_d = nc.dram_tensor("dbgx1n", [128, 256], bf16, kind="ExternalOutput")
        dbggB_d = nc.dram_tensor("dbggB", [65, NPAD], bf16, kind="ExternalOutput")
        dbgzz_d = nc.dram_tensor("dbgzz", [128, 1024], bf16, kind="ExternalOutput")

    reps = int(os.environ.get("KERNEL_REPS", "1"))
    UNROLL = 1
    if reps > 1:
        for u in (8, 4, 2):
            if reps % u == 0:
                UNROLL = u
                break
    STAGGER = os.environ.get("KERNEL_STAGGER", "1") == "1"
    # POOL=1: gpsimd carries SBUF multiplies + halve adds (cost-model says ok).
    # POOL=0: hedge for slow real-HW gpsimd — only partition_broadcast stays.
    POOL = os.environ.get("KERNEL_POOL", "1") == "1"

    with tile.TileContext(nc) as tc:
        with (
            tc.tile_pool(name="cpool", bufs=1) as cpool,
            tc.tile_pool(name="spool", bufs=4) as spool,
            tc.tile_pool(name="pS", bufs=2, space="PSUM") as pS,
            tc.tile_pool(name="pW", bufs=2, space="PSUM") as pW,
            tc.tile_pool(name="pY", bufs=2, space="PSUM") as pY,
        ):
            # first-use order: embT (scores), then batch 0/1 inputs, then weights
            embTall = cpool.tile([16, BPC * NPAD], f32r, name="embTall", tag="embT")
            # split the big f32r load across both hwdge queues
            engs = [nc.sync, nc.scalar, nc.sync, nc.scalar]
            for q in range(4):
                engs[q].dma_start(
                    embTall[:, NPAD * q : NPAD * (q + 1)],
                    embT_d.ap()[:, NPAD * q : NPAD * (q + 1)],
                )
            W1s = cpool.tile([128, 1024], bf16, name="W1s", tag="W1s")
            W2s = cpool.tile([65, 1024], bf16, name="W2s", tag="W2s")
            ids = cpool.tile([128, 64], bf16, name="ids", tag="ids")

            # per-batch tile handles (filled at emission)
            xm_t, gA_t, gB_t, P_t = {}, {}, {}, {}

            def emit_dmas(b):
                xm_t[b] = spool.tile([128, XM_W], bf16, name="xm_s", tag="xm")
                nc.sync.dma_start(xm_t[b], xm_d.ap()[b])
                gA_t[b] = spool.tile([128, NPAD], bf16, name="gA", tag="gA")
                nc.sync.dma_start(gA_t[b][0:64, :], xT_d.ap()[b])
                gB_t[b] = spool.tile([65, NPAD], bf16, name="gB", tag="gB")
                nc.scalar.dma_start(gB_t[b][64:65, :], ones_d.ap())

            def emit_scores_mm(b):
                """scores matmuls for batch b (PE); exps emitted separately."""
                embT_s = embTall[:, NPAD * b : NPAD * (b + 1)]
                P_t[b] = spool.tile([128, 2048], bf16, name="P", tag="P")
                sps = []
                for t in range(NT):
                    Sp = pS.tile([128, 512], f32, name="Sp", tag="S")
                    nc.tensor.matmul(
                        Sp,
                        embT_s[:, 128 * t : 128 * t + 128],
                        embT_s,
                        start=True,
                        stop=True,
                    )
                    sps.append(Sp)
                return sps

            def emit_scores_exp(b, sps):
                for t in range(NT):
                    nc.scalar.activation(
                        P_t[b][:, 512 * t : 512 * t + 512], sps[t], AF.Exp
                    )

            def emit_max_dve(b):
                P = P_t[b]
                nc.vector.tensor_scalar_max(P[:, 0:1024], P[:, 0:1024], 1.0)

            def emit_max_pool(b):
                P = P_t[b]
                nc.gpsimd.tensor_scalar_max(P[:, 1024:2048], P[:, 1024:2048], 1.0)

            def emit_slot(b, nb):
                """Main work for batch b; scores for batch nb (or None)."""
                xm_s, gA, gB, P = xm_t[b], gA_t[b], gB_t[b], P_t[b]

                # ---- u1T = xa.T @ P  [65, 512]; row 64 = r ----
                Ju1 = pW.tile([65, NPAD], f32, name="Ju1", tag="w")
                for c in range(NT):
                    nc.tensor.matmul(
                        Ju1,
                        xm_s[:, XA_OFF + 65 * c : XA_OFF + 65 * c + 65],
                        P[:, 512 * c : 512 * c + 512],
                        start=(c == 0),
                        stop=(c == 3),
                    )

                # scores of next batch fill PE while DVE/Pool make gA
                sps = emit_scores_mm(nb) if nb is not None else None

                rT = spool.tile([1, NPAD], f32, name="rT", tag="rT")
                # NB: reciprocal_approx_fast NaNs on HW for large inputs
                # (row sums reach ~1e13); use the exact iterative divide.
                nc.vector.reciprocal(rT, Ju1[64:65, :])
                rB = spool.tile([64, NPAD], f32, name="rB", tag="rB")
                nc.gpsimd.partition_broadcast(rB, rT)

                if dbg and b == 0:
                    nc.sync.dma_start(dbgJu1_d.ap(), Ju1)
                    nc.sync.dma_start(dbgrB_d.ap(), rB)

                # ---- x_g1T = u1T * rinv -> gA rows 64:127 (DVE: reads PSUM) ----
                nc.vector.tensor_tensor(gA[64:128, :], Ju1[0:64, :], rB, op=ALU.mult)

                # ---- x_g1n = transpose(x_g1T) ----
                Jg1 = pW.tile([128, 256], f32, name="Jg1", tag="w")
                for t in range(NT):
                    nc.tensor.matmul(
                        Jg1[:, 64 * t : 64 * t + 64],
                        gA[64:128, 128 * t : 128 * t + 128],
                        ids[64:128, :],
                        start=True,
                        stop=True,
                    )
                xg1n = spool.tile([128, 256], bf16, name="xg1n", tag="xg1n")
                nc.scalar.copy(xg1n, Jg1)

                # next-batch exps go on ACT *after* the critical xg1n evac
                if nb is not None:
                    emit_scores_exp(nb, sps)

                if dbg and b == 0:
                    nc.sync.dma_start(dbggA_d.ap(), gA)
                    nc.sync.dma_start(dbgx1n_d.ap(), xg1n)

                # ---- u2T = x_g1n.T @ P  [64, 512] ----
                Ju2 = pW.tile([64, NPAD], f32, name="Ju2", tag="w")
                for c in range(NT):
                    kc = NTS[c]
                    nc.tensor.matmul(
                        Ju2,
                        xg1n[0:kc, 64 * c : 64 * c + 64],
                        P[0:kc, 512 * c : 512 * c + 512],
                        start=(c == 0),
                        stop=(c == 3),
                    )

                # ---- gB[0:64] = u2T * rinv  (DVE; -x folded into W1) ----
                nc.vector.tensor_tensor(gB[0:64, :], Ju2, rB, op=ALU.mult)

                if dbg and b == 0:
                    nc.sync.dma_start(dbggB_d.ap(), gB)

                # ---- y = gT.T @ Waug ; z = y * s ; out = sum_e z ----
                # t=0,1: DVE multiplies straight out of PSUM.
                # t=2,3: ACT copies PSUM->SBUF, Pool multiplies in SBUF.
                out_s = spool.tile([128, 256], bf16, name="out_s", tag="out")
                yps = {}

                def y_w1(t):
                    yps[t] = pY.tile([128, 1024], f32, name="yp", tag="y")
                    for fc in range(2):
                        nc.tensor.matmul(
                            yps[t][:, 512 * fc : 512 * fc + 512],
                            gA[:, 128 * t : 128 * t + 128],
                            W1s[:, 512 * fc : 512 * fc + 512],
                            start=True,
                            stop=False,
                        )

                def y_w2_z(t):
                    yp = yps[t]
                    for fc in range(2):
                        nc.tensor.matmul(
                            yp[:, 512 * fc : 512 * fc + 512],
                            gB[:, 128 * t : 128 * t + 128],
                            W2s[:, 512 * fc : 512 * fc + 512],
                            start=False,
                            stop=True,
                        )
                    zz = spool.tile([128, 1024], bf16, name="zz", tag="zz")
                    z3 = zz.rearrange("p (o e) -> p o e", e=16)
                    y3 = yp.rearrange("p (o e) -> p o e", e=16)
                    s3 = (
                        xm_s[:, S_OFF + 16 * t : S_OFF + 16 * t + 16]
                        .rearrange("p (x e) -> p x e", x=1)
                        .broadcast_to((128, 64, 16))
                    )
                    if t < 2:
                        nc.vector.tensor_tensor(z3, y3, s3, op=ALU.mult)
                    else:
                        nc.scalar.copy(zz, yp)
                        nc.gpsimd.tensor_tensor(z3, z3, s3, op=ALU.mult)
                    if dbg and b == 0 and t == 0:
                        nc.sync.dma_start(dbgzz_d.ap(), zz)
                    nc.gpsimd.tensor_tensor(
                        z3[:, :, 0:8], z3[:, :, 0:8], z3[:, :, 8:16], op=ALU.add
                    )
                    nc.gpsimd.tensor_tensor(
                        z3[:, :, 0:4], z3[:, :, 0:4], z3[:, :, 4:8], op=ALU.add
                    )
                    with nc.allow_low_precision("4-term e-tail in bf16"):
                        nc.vector.tensor_reduce(
                            out_s[:, 64 * t : 64 * t + 64],
                            z3[:, :, 0:4],
                            axis=mybir.AxisListType.X,
                            op=ALU.add,
                        )

                y_w1(0)
                y_w1(1)
                y_w2_z(0)
                y_w2_z(1)
                y_w1(2)
                y_w2_z(2)
                y_w1(3)
                y_w2_z(3)

                # P of next batch is final once its exps are done
                if nb is not None:
                    emit_max_dve(nb)
                    emit_max_pool(nb)

                nc.scalar.dma_start(out_d.ap()[b], out_s)

            # ---- prologue: batch 0/1 inputs + batch 0 scores ----
            emit_dmas(0)
            emit_dmas(1)
            nc.sync.dma_start(W1s, W1_d.ap())
            nc.scalar.dma_start(W2s, W2_d.ap())
            nc.sync.dma_start(ids, id_d.ap())
            sps0 = emit_scores_mm(0)
            emit_scores_exp(0, sps0)
            emit_max_dve(0)
            emit_max_pool(0)

            import contextlib

            loop_cm = (
                tc.For_i(0, reps // UNROLL, 1, staggered_reset=STAGGER)
                if reps > 1
                else contextlib.nullcontext()
            )
            with loop_cm:
                nslots = BPC * UNROLL
                for s in range(nslots):
                    b = s % BPC
                    pf = (b + 2) % BPC  # input prefetch, 2 slots ahead
                    if reps > 1 or s + 2 < nslots:
                        emit_dmas(pf)
                    nb = (b + 1) % BPC if (reps > 1 or s + 1 < nslots) else None
                    emit_slot(b, nb)

    nc.compile()
    _CACHE["nc"] = nc
    return nc


def _prep_inputs(x, all_emb, station_emb, weights_pool, bias_pool):
    """Host-side layout prep. Returns in_maps (one dict per core)."""
    x = np.asarray(x, np.float32)
    all_emb = np.asarray(all_emb, np.float32)
    station_emb = np.asarray(station_emb, np.float32)
    weights_pool = np.asarray(weights_pool, np.float32)
    bias_pool = np.asarray(bias_pool, np.float32)

    # W'[k*64+i, o*16+e] = weights_pool[e, k, i, o]
    Wp = np.transpose(weights_pool, (1, 2, 3, 0))  # [k, i, o, e]
    W1 = np.empty((128, 1024), np.float32)
    # Cheb: x_g2 = 2*A(Ax) - x; the -x term folds into the x weights
    W1[0:64] = (Wp[0] - Wp[2]).reshape(64, 1024)
    W1[64:128] = Wp[1].reshape(64, 1024)
    W1 = W1.astype(BF16)
    W2 = np.empty((65, 1024), np.float32)
    W2[0:64] = 2.0 * Wp[2].reshape(64, 1024)  # applied to A(Ax)
    W2[64] = np.transpose(bias_pool, (1, 0)).reshape(1024)  # ones row -> bias
    W2 = W2.astype(BF16)
    ident = np.tile(np.eye(64, dtype=np.float32), (2, 1)).astype(BF16)
    onesrow = np.ones((1, NPAD), np.float32).astype(BF16)

    node_valid = (
        np.arange(4)[:, None] * 128 + np.arange(128)[None, :]
    ) < N  # [4, 128]

    in_maps = []
    for core in range(NCORES):
        b0 = core * BPC
        xb = x[b0 : b0 + BPC]  # [4, 500, 64]
        eb = all_emb[b0 : b0 + BPC]
        sb = station_emb[b0 : b0 + BPC]

        embT = np.zeros((BPC, 16, NPAD), np.float32)
        embT[:, :, 0:N] = np.transpose(eb, (0, 2, 1))
        embT = np.transpose(embT, (1, 0, 2)).reshape(16, BPC * NPAD)

        xpad = np.zeros((BPC, NPAD, DIN), np.float32)
        xpad[:, 0:N] = xb
        xch = np.transpose(xpad.reshape(BPC, 4, 128, DIN), (0, 2, 1, 3))

        xm = np.zeros((BPC, 128, XM_W), np.float32)
        xa = np.concatenate(
            [
                xch,
                np.broadcast_to(
                    np.transpose(node_valid, (1, 0))[None, :, :, None],
                    (BPC, 128, 4, 1),
                ).astype(np.float32),
            ],
            axis=3,
        )  # [BPC, 128, 4, 65]
        xm[:, :, XA_OFF : XA_OFF + 260] = xa.reshape(BPC, 128, 260)
        spad = np.zeros((BPC, NPAD, E), np.float32)
        spad[:, 0:N] = sb
        xm[:, :, S_OFF : S_OFF + 64] = np.transpose(
            spad.reshape(BPC, 4, 128, E), (0, 2, 1, 3)
        ).reshape(BPC, 128, 64)

        xT = np.zeros((BPC, 64, NPAD), np.float32)
        xT[:, :, 0:N] = np.transpose(xb, (0, 2, 1))

        in_maps.append(
            {
                "embT": embT,
                "xmisc": xm.astype(BF16),
                "xT": xT.astype(BF16),
                "onesrow": onesrow,
                "W1": W1,
                "W2": W2,
                "ident": ident,
            }
        )
    return in_maps


def _gather(results):
    """results: list of per-core dicts with 'out' [BPC, 128, 256] bf16."""
    out = np.zeros((B, N, DOUT), np.float32)
    for core in range(NCORES):
        r = np.asarray(results[core]["out"], dtype=np.float32)  # [4,128,256]
        r = r.reshape(BPC, 128, 4, 64)
        for t in range(NT):
            nt = NTS[t]
            out[core * BPC : (core + 1) * BPC, 128 * t : 128 * t + nt, :] = r[
                :, 0:nt, t, :
            ]
    return out


def kernel(_trace=False, _trace_kwargs=None, **inputs):
    nc = _build_program()
    in_maps = _prep_inputs(**inputs)
    res = run_bass_kernel_spmd(
        nc,
        in_maps,
        core_ids=list(range(NCORES)),
        trace=_trace,
        **(_trace_kwargs or {}),
    )
    _CACHE["last_result"] = res
    return _gather(res.results)
